# revision 15
# baseline (speedup 1.0000x reference)
"""Trainium2 Bass kernel for nn_GATOnlyRecommender (2-layer GAT + MLP predictor).

Strategy (self-contained, 8 NeuronCores, no collectives):
  The batch of 4096 (user, business) pairs is split into 8 groups of 512
  pairs (one per core).  Because the node-feature matrix x is zero except at
  the batch nodes, the two GAT layers can be evaluated demand-driven on a
  small neighborhood closure of each core's pairs:

    P  = this core's pair nodes (<=1024)
    EP = graph edges into P (+ self loops)          -> layer-2 edge set
    Q  = srcs of EP  (union P)                      -> nodes needing x1/h2
    EQP= edges into Q whose src is a *touched* node -> layer-1 message edges
    N1 = touched nodes needing h1/a1 (srcs of EQP, union P_glob & Q)

  Layer-1 softmax denominators use the identity: every edge into q whose
  src is untouched contributes exp(lrelu(a1d[q])); so
     denom[q] = plain_cnt[q]*exp(lrelu(a1d[q])) + sum_{EQP into q} exp(e).
  (Segment-max subtraction is skipped: softmax is shift-invariant and the
  logits here are O(0.1).)  Normalization uses a reciprocal table gathered
  per edge; biases fold into per-partition activation bias.

  On device, per-edge values come from dma_gather row-gathers over small
  DRAM tables; segment sums are one-hot matmuls (R[e, q] built by an iota
  compare) accumulated per 128-node block in PSUM.

  Host-side work is limited to index/partition prep (graph partitioning per
  the sharding hint) — all FLOPs and tensor movement happen on device.
"""

import os
import sys

for _p in ("/opt/trn_rl_repo",):
    if _p not in sys.path:
        sys.path.insert(0, _p)

import numpy as np

import concourse.bacc as bacc
import concourse.mybir as mybir
import concourse.tile as tile
from concourse.bass import AP, IndirectOffsetOnAxis
from concourse.bass_utils import run_bass_kernel_spmd

F32 = mybir.dt.float32
I16 = mybir.dt.int16
I32 = mybir.dt.int32
AF = mybir.ActivationFunctionType
ALU = mybir.AluOpType

NU = 40000          # users
NB = 60000          # businesses
NNODES = NU + NB
H = 128             # hidden
HEADS = 4
SLOPE = 0.2
NCORES = 8
BATCH = 4096
PAIRS = BATCH // NCORES      # 512 pairs per core
EPS = 1e-16


def ts(i, size):
    return slice(i * size, (i + 1) * size)


class Caps:
    """Static capacities (all multiples of 128)."""

    def __init__(self, n1u_t=7, n1b_t=7, qb=28, eqp_tpb=1, ep_tpb=4):
        self.N1U_T = n1u_t            # user xc tiles (128 rows each)
        self.N1B_T = n1b_t
        self.N1_T = n1u_t + n1b_t
        self.N1_CAP = self.N1_T * 128
        self.QB = qb                  # q blocks of 128
        self.Q_CAP = qb * 128
        self.EQP_TPB = eqp_tpb        # EQP tiles per q block
        self.EQP_T = qb * eqp_tpb
        self.EQP_CAP = self.EQP_T * 128
        self.EP_TPB = ep_tpb          # EP tiles per p block
        self.PB = 8                   # p blocks (P_CAP = 1024)
        self.P_CAP = self.PB * 128
        self.EP_T = self.PB * ep_tpb
        self.EP_CAP = self.EP_T * 128

    def key(self):
        return (self.N1U_T, self.N1B_T, self.QB, self.EQP_TPB, self.EP_TPB)


class CapacityError(Exception):
    pass


# ----------------------------------------------------------------------------
# Device program
# ----------------------------------------------------------------------------
def build_nc(caps: Caps, phase_limit=9):
    c = caps
    nc = bacc.Bacc("TRN2", target_bir_lowering=False, debug=False,
                   num_devices=NCORES)

    def inp(name, shape, dt=F32):
        return nc.dram_tensor(name, list(shape), dt, kind="ExternalInput").ap()

    user_table = inp("user_table", (NU, H))
    business_table = inp("business_table", (NB, H))
    W1 = inp("W1", (128, 512))            # [k, h*c]
    W1T = inp("W1T", (128, 512))          # [c, h*k]
    W2cm = inp("W2cm", (128, 512))        # [k, c*o]  chunk-major
    W2T = inp("W2T", (128, 512))          # [o, c*k]
    Wp1cm = inp("Wp1cm", (128, 256))      # [ch, c*h]
    Wp2 = inp("Wp2", (128, 1))
    att1p = inp("att1p", (128, 8))        # [c, (h, src/dst)]
    att2p = inp("att2p", (128, 2))        # [o, (src,dst)]
    b1c = inp("b1c", (128, 4))            # b1 chunks as columns
    b2r = inp("b2r", (1, 128))
    bp1c = inp("bp1c", (128, 1))
    bp2b = inp("bp2b", (128, 1))
    ones1 = inp("ones1", (1, 128))
    iota2d = inp("iota2d", (128, 128))
    identity = inp("identity", (128, 128))

    xc_u_rows = inp("xc_u_rows", (128, c.N1U_T), I32)
    xc_b_rows = inp("xc_b_rows", (128, c.N1B_T), I32)
    eqp_src16 = inp("eqp_src16", (128, c.EQP_CAP // 16), I16)
    eqp_dst16 = inp("eqp_dst16", (128, c.EQP_CAP // 16), I16)
    eqp_dstq16 = inp("eqp_dstq16", (128, c.EQP_CAP // 16), I16)
    eqp_cnt = inp("eqp_cnt", (128, c.EQP_T))
    eqp_rel = inp("eqp_rel", (128, c.EQP_T))
    virt_dst16 = inp("virt_dst16", (128, c.Q_CAP // 16), I16)
    virt_cnt = inp("virt_cnt", (128, c.QB))
    ep_src16 = inp("ep_src16", (128, c.EP_CAP // 16), I16)
    ep_dstq16 = inp("ep_dstq16", (128, c.EP_CAP // 16), I16)
    ep_cnt = inp("ep_cnt", (128, c.EP_T))
    ep_rel = inp("ep_rel", (128, c.EP_T))
    u_pos16 = inp("u_pos16", (128, PAIRS // 16), I16)
    b_pos16 = inp("b_pos16", (128, PAIRS // 16), I16)

    out = nc.dram_tensor("out", [PAIRS, 1], F32, kind="ExternalOutput").ap()

    h1_table = nc.dram_tensor("h1_table", [c.N1_CAP + 1, 512], F32).ap()
    a1_table = nc.dram_tensor("a1_table", [c.N1_CAP + 1, 64], F32).ap()
    recd_table = nc.dram_tensor("recd_table", [c.Q_CAP, 64], F32).ap()
    h2_table = nc.dram_tensor("h2_table", [c.Q_CAP, 192], F32).ap()
    x2_table = nc.dram_tensor("x2_table", [c.P_CAP, H], F32).ap()

    with tile.TileContext(nc) as tc:
        with tc.tile_pool(name="const", bufs=1) as cp, \
             tc.tile_pool(name="persist", bufs=1) as pp:

            def load(ap_in, shape, dt=F32):
                t = cp.tile(list(shape), dt, tag="c_" + ap_in.tensor.name)
                nc.sync.dma_start(out=t[:], in_=ap_in)
                return t

            W1_sb = load(W1[:, :], (128, 512))
            W1T_sb = load(W1T[:, :], (128, 512))
            W2cm_sb = load(W2cm[:, :], (128, 512))
            W2T_sb = load(W2T[:, :], (128, 512))
            Wp1_sb = load(Wp1cm[:, :], (128, 256))
            Wp2_sb = load(Wp2[:, :], (128, 1))
            att1p_sb = load(att1p[:, :], (128, 8))
            att2p_sb = load(att2p[:, :], (128, 2))
            b1c_sb = load(b1c[:, :], (128, 4))
            b2r_sb = load(b2r[:, :], (1, 128))
            bp1c_sb = load(bp1c[:, :], (128, 1))
            bp2b_sb = load(bp2b[:, :], (128, 1))
            ones1_sb = load(ones1[:, :], (1, 128))
            iota_sb = load(iota2d[:, :], (128, 128))
            ident_sb = load(identity[:, :], (128, 128))

            xcu_idx = load(xc_u_rows[:, :], (128, c.N1U_T), I32)
            xcb_idx = load(xc_b_rows[:, :], (128, c.N1B_T), I32)

            eqp_src_sb = load(eqp_src16[:, :], (128, c.EQP_CAP // 16), I16)
            eqp_dst_sb = load(eqp_dst16[:, :], (128, c.EQP_CAP // 16), I16)
            eqp_dstq_sb = load(eqp_dstq16[:, :], (128, c.EQP_CAP // 16), I16)
            virt_dst_sb = load(virt_dst16[:, :], (128, c.Q_CAP // 16), I16)
            ep_src_sb = load(ep_src16[:, :], (128, c.EP_CAP // 16), I16)
            ep_dstq_sb = load(ep_dstq16[:, :], (128, c.EP_CAP // 16), I16)
            upos_sb = load(u_pos16[:, :], (128, PAIRS // 16), I16)
            bpos_sb = load(b_pos16[:, :], (128, PAIRS // 16), I16)

            eqp_cnt_sb = load(eqp_cnt[:, :], (128, c.EQP_T))
            eqp_rel_sb = load(eqp_rel[:, :], (128, c.EQP_T))
            virt_cnt_sb = load(virt_cnt[:, :], (128, c.QB))
            ep_cnt_sb = load(ep_cnt[:, :], (128, c.EP_T))
            ep_rel_sb = load(ep_rel[:, :], (128, c.EP_T))

            # persistent intermediates
            b2b_sb = pp.tile([128, 128], F32, tag="b2b")
            xcT = pp.tile([128, c.N1_T * 128], F32, tag="xcT")
            R_sb = pp.tile([128, c.EQP_T * 128], F32, tag="R")
            exq_sb = pp.tile([128, c.EQP_T * 4], F32, tag="exq")
            exv_sb = pp.tile([128, c.QB * 4], F32, tag="exv")
            x1T = pp.tile([128, 4 * c.Q_CAP], F32, tag="x1T")
            x2_sb = pp.tile([128, c.PB * H], F32, tag="x2")
            recd_sb = pp.tile([128, c.QB * 4], F32, tag="recd")
            R2_sb = pp.tile([128, c.EP_T * 128], F32, tag="R2")
            wa1_sb = pp.tile([128, 8], F32, tag="wa1")
            w2a_sb = pp.tile([128, 8], F32, tag="w2a")

            exq = exq_sb[:].rearrange("p (t f) -> p t f", f=4)

            # ---------------- phase 0: small precomputes ----------------
            with tc.tile_pool(name="ps0", bufs=1, space="PSUM") as ps0:
                p_b2 = ps0.tile([128, 128], F32, tag="pb2")
                nc.tensor.matmul(out=p_b2[:], lhsT=ones1_sb[:], rhs=b2r_sb[:],
                                 start=True, stop=True)
                nc.vector.tensor_copy(out=b2b_sb[:], in_=p_b2[:])

                zt = cp.tile([1, 512], F32, tag="zt")
                nc.gpsimd.memset(zt[:], 0.0)
                nc.sync.dma_start(out=h1_table[c.N1_CAP:c.N1_CAP + 1, :],
                                  in_=zt[:, 0:512])
                nc.sync.dma_start(out=a1_table[c.N1_CAP:c.N1_CAP + 1, :],
                                  in_=zt[:, 0:64])

                p_wa1 = ps0.tile([128, 8], F32, tag="pwa1")
                for h in range(HEADS):
                    nc.tensor.matmul(out=p_wa1[:, ts(h, 2)],
                                     lhsT=W1T_sb[:, ts(h, 128)],
                                     rhs=att1p_sb[:, ts(h, 2)],
                                     start=True, stop=True)
                nc.vector.tensor_copy(out=wa1_sb[:], in_=p_wa1[:])

                p_w2a = ps0.tile([128, 8], F32, tag="pw2a")
                for ch in range(4):
                    nc.tensor.matmul(out=p_w2a[:, ts(ch, 2)],
                                     lhsT=W2T_sb[:, ts(ch, 128)],
                                     rhs=att2p_sb[:, :],
                                     start=True, stop=True)
                nc.vector.tensor_copy(out=w2a_sb[:], in_=p_w2a[:])

            # ---------------- phase 1+2: x_c gather, h1/a1 ----------------
            if phase_limit >= 2:
                with tc.tile_pool(name="w12", bufs=3) as wp, \
                     tc.tile_pool(name="ps12", bufs=2, space="PSUM") as ps:
                    for t in range(c.N1_T):
                        xr = wp.tile([128, H], F32, tag="xcg")
                        if t < c.N1U_T:
                            off = IndirectOffsetOnAxis(
                                ap=xcu_idx[:, t:t + 1], axis=0)
                            nc.gpsimd.indirect_dma_start(
                                out=xr[:], out_offset=None,
                                in_=user_table[:, :], in_offset=off)
                        else:
                            off = IndirectOffsetOnAxis(
                                ap=xcb_idx[:, t - c.N1U_T:t - c.N1U_T + 1],
                                axis=0)
                            nc.gpsimd.indirect_dma_start(
                                out=xr[:], out_offset=None,
                                in_=business_table[:, :], in_offset=off)
                        p_t = ps.tile([128, 128], F32, tag="ptr")
                        nc.tensor.transpose(out=p_t[:], in_=xr[:],
                                            identity=ident_sb[:])
                        nc.vector.tensor_copy(out=xcT[:, ts(t, 128)],
                                              in_=p_t[:])

                    for t in range(c.N1_T):
                        p_h1 = ps.tile([128, 512], F32, tag="ph1")
                        nc.tensor.matmul(out=p_h1[:], lhsT=xcT[:, ts(t, 128)],
                                         rhs=W1_sb[:], start=True, stop=True)
                        h1t = wp.tile([128, 512], F32, tag="h1w")
                        nc.vector.tensor_copy(out=h1t[:], in_=p_h1[:])
                        nc.sync.dma_start(out=h1_table[ts(t, 128), :],
                                          in_=h1t[:])

                        p_a1 = ps.tile([128, 8], F32, tag="pa1")
                        nc.tensor.matmul(out=p_a1[:], lhsT=xcT[:, ts(t, 128)],
                                         rhs=wa1_sb[:], start=True, stop=True)
                        a1t = wp.tile([128, 8], F32, tag="a1w")
                        nc.vector.tensor_copy(out=a1t[:], in_=p_a1[:])
                        nc.sync.dma_start(out=a1_table[ts(t, 128), 0:8],
                                          in_=a1t[:])

            # ---------------- phase 3: per-edge exp + denominators --------
            if phase_limit >= 3:
                with tc.tile_pool(name="weq", bufs=1) as wq, \
                     tc.tile_pool(name="wden", bufs=2) as wd, \
                     tc.tile_pool(name="psden", bufs=3, space="PSUM") as psd:
                    agv = wq.tile([128, c.QB, 64], F32, tag="agv")
                    nc.gpsimd.dma_gather(
                        out_ap=agv[:, :, :], in_ap=a1_table[:, :],
                        idxs_ap=virt_dst_sb[:, :], num_idxs=c.Q_CAP,
                        num_idxs_reg=c.Q_CAP, elem_size=64, single_packet=False)
                    ags = wq.tile([128, c.EQP_T, 64], F32, tag="ags")
                    agd = wq.tile([128, c.EQP_T, 64], F32, tag="agd")
                    nc.gpsimd.dma_gather(
                        out_ap=ags[:, :, :], in_ap=a1_table[:, :],
                        idxs_ap=eqp_src_sb[:, :], num_idxs=c.EQP_CAP,
                        num_idxs_reg=c.EQP_CAP, elem_size=64,
                        single_packet=False)
                    nc.gpsimd.dma_gather(
                        out_ap=agd[:, :, :], in_ap=a1_table[:, :],
                        idxs_ap=eqp_dst_sb[:, :], num_idxs=c.EQP_CAP,
                        num_idxs_reg=c.EQP_CAP, elem_size=64,
                        single_packet=False)
                    ev = wq.tile([128, c.QB, 4], F32, tag="ev")
                    evt = wq.tile([128, c.QB, 4], F32, tag="evt")
                    for h in range(HEADS):
                        nc.vector.tensor_copy(
                            out=ev[:, :, h:h + 1],
                            in_=agv[:, :, 2 * h + 1:2 * h + 2])
                    nc.vector.tensor_scalar_mul(out=evt[:, :, :],
                                                in0=ev[:, :, :],
                                                scalar1=SLOPE)
                    nc.vector.tensor_tensor(out=ev[:, :, :], in0=ev[:, :, :],
                                            in1=evt[:, :, :], op=ALU.max)
                    nc.scalar.activation(out=ev[:, :, :], in_=ev[:, :, :],
                                         func=AF.Exp)
                    nc.vector.tensor_tensor(
                        out=exv_sb[:].rearrange("p (b f) -> p b f", f=4),
                        in0=ev[:, :, :],
                        in1=virt_cnt_sb[:, :].to_broadcast([128, c.QB, 4]),
                        op=ALU.mult)

                    for h in range(HEADS):
                        nc.vector.tensor_tensor(
                            out=exq[:, :, h:h + 1],
                            in0=ags[:, :, 2 * h:2 * h + 1],
                            in1=agd[:, :, 2 * h + 1:2 * h + 2],
                            op=ALU.add)
                    exqt = wq.tile([128, c.EQP_T, 4], F32, tag="exqt")
                    nc.vector.tensor_scalar_mul(out=exqt[:, :, :],
                                                in0=exq[:, :, :],
                                                scalar1=SLOPE)
                    nc.vector.tensor_tensor(out=exq[:, :, :],
                                            in0=exq[:, :, :],
                                            in1=exqt[:, :, :], op=ALU.max)
                    nc.scalar.activation(out=exq[:, :, :], in_=exq[:, :, :],
                                         func=AF.Exp)
                    nc.vector.tensor_tensor(
                        out=exq[:, :, :], in0=exq[:, :, :],
                        in1=eqp_cnt_sb[:, :].to_broadcast([128, c.EQP_T, 4]),
                        op=ALU.mult)

                    for t in range(c.EQP_T):
                        nc.vector.tensor_tensor(
                            out=R_sb[:, ts(t, 128)],
                            in0=eqp_rel_sb[:, t:t + 1].to_broadcast(
                                [128, 128]),
                            in1=iota_sb[:, :], op=ALU.is_equal)

                    for b in range(c.QB):
                        p_den = psd.tile([128, 4], F32, tag="pden")
                        for i in range(c.EQP_TPB):
                            t = b * c.EQP_TPB + i
                            nc.tensor.matmul(out=p_den[:],
                                             lhsT=R_sb[:, ts(t, 128)],
                                             rhs=exq[:, t, :],
                                             start=(i == 0),
                                             stop=(i == c.EQP_TPB - 1))
                        dtot = wd.tile([128, 4], F32, tag="dtot")
                        nc.vector.tensor_tensor(out=dtot[:], in0=p_den[:],
                                                in1=exv_sb[:, ts(b, 4)],
                                                op=ALU.add)
                        nc.vector.tensor_scalar_add(out=dtot[:], in0=dtot[:],
                                                    scalar1=EPS)
                        nc.vector.reciprocal(out=recd_sb[:, ts(b, 4)],
                                             in_=dtot[:])
                    nc.sync.dma_start(
                        out=AP(recd_table.tensor, 0,
                               [[64, 128], [128 * 64, c.QB], [1, 4]]),
                        in_=recd_sb[:].rearrange("p (b f) -> p b f", f=4))

            # ---------------- phase 4: messages -> x1T ----------------
            if phase_limit >= 4:
                CH = 4  # q blocks per chunk
                with tc.tile_pool(name="wmsg", bufs=3) as wm, \
                     tc.tile_pool(name="psx1", bufs=3, space="PSUM") as psx:
                    for c0 in range(0, c.QB, CH):
                        nt = CH * c.EQP_TPB
                        t0 = c0 * c.EQP_TPB
                        h1g = wm.tile([128, nt, 512], F32, tag="h1g")
                        rcg = wm.tile([128, nt, 64], F32, tag="rcg")
                        nc.gpsimd.dma_gather(
                            out_ap=h1g[:, :, :], in_ap=h1_table[:, :],
                            idxs_ap=eqp_src_sb[:, t0 * 8:(t0 + nt) * 8],
                            num_idxs=nt * 128, num_idxs_reg=nt * 128,
                            elem_size=512, single_packet=False)
                        nc.gpsimd.dma_gather(
                            out_ap=rcg[:, :, :], in_ap=recd_table[:, :],
                            idxs_ap=eqp_dstq_sb[:, t0 * 8:(t0 + nt) * 8],
                            num_idxs=nt * 128, num_idxs_reg=nt * 128,
                            elem_size=64, single_packet=False)
                        alpha = wm.tile([128, nt, 4], F32, tag="alpha")
                        nc.vector.tensor_tensor(
                            out=alpha[:, :, :], in0=exq[:, t0:t0 + nt, :],
                            in1=rcg[:, :, 0:4], op=ALU.mult)
                        mv = wm.tile([128, nt, 512], F32, tag="mv")
                        nc.vector.tensor_tensor(
                            out=mv[:, :, :].rearrange(
                                "p t (h f) -> p t h f", h=4),
                            in0=h1g[:, :, :].rearrange(
                                "p t (h f) -> p t h f", h=4),
                            in1=alpha[:, :, :].to_broadcast(
                                [128, nt, 4, 128]),
                            op=ALU.mult)
                        for bi in range(CH):
                            b = c0 + bi
                            p_x1 = psx.tile([128, 512], F32, tag="px1")
                            for ch in range(4):
                                for i in range(c.EQP_TPB):
                                    t = b * c.EQP_TPB + i
                                    ti = bi * c.EQP_TPB + i
                                    nc.tensor.matmul(
                                        out=p_x1[:, ts(ch, 128)],
                                        lhsT=mv[:, ti, ts(ch, 128)],
                                        rhs=R_sb[:, ts(t, 128)],
                                        start=(i == 0),
                                        stop=(i == c.EQP_TPB - 1))
                            for ch in range(4):
                                nc.scalar.activation(
                                    out=x1T[:, ch * c.Q_CAP + b * 128:
                                            ch * c.Q_CAP + (b + 1) * 128],
                                    in_=p_x1[:, ts(ch, 128)],
                                    func=AF.Relu, bias=b1c_sb[:, ch:ch + 1])

            # ---------------- phase 5: h2 / a2 ----------------
            if phase_limit >= 5:
                with tc.tile_pool(name="wh2", bufs=3) as wh, \
                     tc.tile_pool(name="psh2", bufs=3, space="PSUM") as psh:
                    for t in range(c.QB):
                        p_h2 = psh.tile([128, H], F32, tag="ph2")
                        p_a2 = psh.tile([128, 2], F32, tag="pa2")
                        for ch in range(4):
                            lhs = x1T[:, ch * c.Q_CAP + t * 128:
                                      ch * c.Q_CAP + (t + 1) * 128]
                            nc.tensor.matmul(out=p_h2[:], lhsT=lhs,
                                             rhs=W2cm_sb[:, ts(ch, 128)],
                                             start=(ch == 0), stop=(ch == 3))
                            nc.tensor.matmul(out=p_a2[:], lhsT=lhs,
                                             rhs=w2a_sb[:, ts(ch, 2)],
                                             start=(ch == 0), stop=(ch == 3))
                        h2t = wh.tile([128, H], F32, tag="h2w")
                        nc.vector.tensor_copy(out=h2t[:], in_=p_h2[:])
                        nc.sync.dma_start(out=h2_table[ts(t, 128), 0:H],
                                          in_=h2t[:])
                        a2t = wh.tile([128, 2], F32, tag="a2w")
                        nc.vector.tensor_copy(out=a2t[:], in_=p_a2[:])
                        nc.sync.dma_start(out=h2_table[ts(t, 128), 128:130],
                                          in_=a2t[:])

            # ---------------- phase 6: layer-2 edges -> x2 ----------------
            if phase_limit >= 6:
                with tc.tile_pool(name="wep", bufs=1) as we, \
                     tc.tile_pool(name="wep2", bufs=4) as we2, \
                     tc.tile_pool(name="psx2", bufs=3, space="PSUM") as psp:
                    for t in range(c.EP_T):
                        nc.vector.tensor_tensor(
                            out=R2_sb[:, ts(t, 128)],
                            in0=ep_rel_sb[:, t:t + 1].to_broadcast(
                                [128, 128]),
                            in1=iota_sb[:, :], op=ALU.is_equal)
                    cg = we.tile([128, c.EP_T, 192], F32, tag="cg")
                    agd2 = we.tile([128, c.EP_T, 192], F32, tag="agd2")
                    nc.gpsimd.dma_gather(
                        out_ap=cg[:, :, :], in_ap=h2_table[:, :],
                        idxs_ap=ep_src_sb[:, :], num_idxs=c.EP_CAP,
                        num_idxs_reg=c.EP_CAP, elem_size=192,
                        single_packet=False)
                    nc.gpsimd.dma_gather(
                        out_ap=agd2[:, :, :], in_ap=h2_table[:, :],
                        idxs_ap=ep_dstq_sb[:, :], num_idxs=c.EP_CAP,
                        num_idxs_reg=c.EP_CAP, elem_size=192,
                        single_packet=False)
                    ex2 = we.tile([128, c.EP_T], F32, tag="ex2")
                    ex2v = ex2[:, :].rearrange("p (t o) -> p t o", o=1)
                    nc.vector.tensor_tensor(out=ex2v, in0=cg[:, :, 128:129],
                                            in1=agd2[:, :, 129:130], op=ALU.add)
                    ex2t = we.tile([128, c.EP_T], F32, tag="ex2t")
                    nc.vector.tensor_scalar_mul(out=ex2t[:, :], in0=ex2[:, :],
                                                scalar1=SLOPE)
                    nc.vector.tensor_tensor(out=ex2[:, :], in0=ex2[:, :],
                                            in1=ex2t[:, :], op=ALU.max)
                    nc.scalar.activation(out=ex2v, in_=ex2v, func=AF.Exp)
                    nc.vector.tensor_tensor(out=ex2[:, :], in0=ex2[:, :],
                                            in1=ep_cnt_sb[:, :], op=ALU.mult)
                    mv2 = we.tile([128, c.EP_T, 132], F32, tag="mv2")
                    nc.gpsimd.memset(mv2[:], 0.0)
                    nc.vector.tensor_tensor(
                        out=mv2[:, :, 0:H], in0=cg[:, :, 0:H],
                        in1=ex2[:, :].to_broadcast([128, c.EP_T, H]),
                        op=ALU.mult)
                    nc.vector.tensor_copy(out=mv2[:, :, 128:129], in_=ex2v)

                    for b in range(c.PB):
                        p_x2 = psp.tile([128, 132], F32, tag="px2")
                        for i in range(c.EP_TPB):
                            t = b * c.EP_TPB + i
                            nc.tensor.matmul(out=p_x2[:],
                                             lhsT=R2_sb[:, ts(t, 128)],
                                             rhs=mv2[:, t, :],
                                             start=(i == 0),
                                             stop=(i == c.EP_TPB - 1))
                        rcd2 = we2.tile([128, 1], F32, tag="rcd2")
                        nc.vector.tensor_scalar_add(out=rcd2[:],
                                                    in0=p_x2[:, 128:129],
                                                    scalar1=EPS)
                        nc.vector.reciprocal(out=rcd2[:], in_=rcd2[:])
                        nc.scalar.activation(out=x2_sb[:, ts(b, H)],
                                             in_=p_x2[:, 0:H],
                                             func=AF.Copy,
                                             scale=rcd2[:, 0:1])
                        nc.vector.tensor_tensor(out=x2_sb[:, ts(b, H)],
                                                in0=x2_sb[:, ts(b, H)],
                                                in1=b2b_sb[:], op=ALU.add)
                    nc.sync.dma_start(
                        out=AP(x2_table.tensor, 0,
                               [[H, 128], [128 * H, c.PB], [1, H]]),
                        in_=x2_sb[:].rearrange("p (b f) -> p b f", f=H))

            # ---------------- phase 7: predictor ----------------
            if phase_limit >= 7:
                with tc.tile_pool(name="wpr", bufs=2) as wr, \
                     tc.tile_pool(name="pspr", bufs=2, space="PSUM") as psr:
                    fu = wr.tile([128, PAIRS // 128, H], F32, tag="fu")
                    fb = wr.tile([128, PAIRS // 128, H], F32, tag="fb")
                    nc.gpsimd.dma_gather(
                        out_ap=fu[:, :, :], in_ap=x2_table[:, :],
                        idxs_ap=upos_sb[:, :], num_idxs=PAIRS,
                        num_idxs_reg=PAIRS, elem_size=H, single_packet=False)
                    nc.gpsimd.dma_gather(
                        out_ap=fb[:, :, :], in_ap=x2_table[:, :],
                        idxs_ap=bpos_sb[:, :], num_idxs=PAIRS,
                        num_idxs_reg=PAIRS, elem_size=H, single_packet=False)
                    for t in range(PAIRS // 128):
                        fTu = wr.tile([128, 128], F32, tag="fTu")
                        fTb = wr.tile([128, 128], F32, tag="fTb")
                        p_tr = psr.tile([128, 128], F32, tag="ptr2")
                        nc.tensor.transpose(out=p_tr[:], in_=fu[:, t, :],
                                            identity=ident_sb[:])
                        nc.vector.tensor_copy(out=fTu[:], in_=p_tr[:])
                        p_tr2 = psr.tile([128, 128], F32, tag="ptr2")
                        nc.tensor.transpose(out=p_tr2[:], in_=fb[:, t, :],
                                            identity=ident_sb[:])
                        nc.vector.tensor_copy(out=fTb[:], in_=p_tr2[:])

                        p_hd = psr.tile([128, 128], F32, tag="phd")
                        nc.tensor.matmul(out=p_hd[:],
                                         lhsT=Wp1_sb[:, ts(0, 128)],
                                         rhs=fTu[:], start=True, stop=False)
                        nc.tensor.matmul(out=p_hd[:],
                                         lhsT=Wp1_sb[:, ts(1, 128)],
                                         rhs=fTb[:], start=False, stop=True)
                        hT = wr.tile([128, 128], F32, tag="hT")
                        nc.scalar.activation(out=hT[:], in_=p_hd[:],
                                             func=AF.Relu,
                                             bias=bp1c_sb[:, 0:1])
                        p_r = psr.tile([128, 1], F32, tag="pr")
                        nc.tensor.matmul(out=p_r[:], lhsT=hT[:],
                                         rhs=Wp2_sb[:],
                                         start=True, stop=True)
                        r_sb = wr.tile([128, 1], F32, tag="rsb")
                        nc.scalar.activation(out=r_sb[:], in_=p_r[:],
                                             func=AF.Identity,
                                             bias=bp2b_sb[:, 0:1])
                        nc.sync.dma_start(out=out[ts(t, 128), :],
                                          in_=r_sb[:, 0:1])

    nc.compile()
    return nc


# ----------------------------------------------------------------------------
# Host-side prep
# ----------------------------------------------------------------------------
def _pack16(vals, cap, pad=0):
    a = np.full(cap, pad, np.int64)
    a[:len(vals)] = vals
    assert a.max() < 32768
    blk = a.astype(np.int16).reshape(cap // 16, 16).T
    return np.ascontiguousarray(np.tile(blk, (8, 1)))


def _slotmaj(vals, cap, pad=0.0):
    a = np.full(cap, pad, np.float32)
    a[:len(vals)] = vals
    return np.ascontiguousarray(a.reshape(cap // 128, 128).T)


def prep_core(core, user_idx, business_idx, es, ed, caps: Caps):
    c = caps
    sl = slice(core * PAIRS, (core + 1) * PAIRS)
    u_nodes = user_idx[sl]
    b_nodes = business_idx[sl]

    P_glob = np.unique(np.concatenate([user_idx, business_idx]))
    P = np.unique(np.concatenate([u_nodes, b_nodes]))
    if len(P) > c.P_CAP:
        raise CapacityError("P")

    in_P = np.isin(ed, P)
    ep_s = np.concatenate([es[in_P], P])
    ep_d = np.concatenate([ed[in_P], P])
    Q = np.unique(np.concatenate([ep_s, P]))
    if len(Q) > c.Q_CAP:
        raise CapacityError("Q")

    in_Q = np.isin(ed, Q)
    eq_s = np.concatenate([es[in_Q], Q])
    eq_d = np.concatenate([ed[in_Q], Q])
    eq_p_mask = np.isin(eq_s, P_glob)
    eqp_s, eqp_d = eq_s[eq_p_mask], eq_d[eq_p_mask]
    plain_d = eq_d[~eq_p_mask]
    plain_cnt = np.zeros(c.Q_CAP, np.float32)
    np.add.at(plain_cnt, np.searchsorted(Q, plain_d), 1.0)

    PQ = np.intersect1d(P_glob, Q)
    N1 = np.unique(np.concatenate([eqp_s, PQ]))
    N1u = N1[N1 < NU]
    N1b = N1[N1 >= NU]
    if len(N1u) > c.N1U_T * 128 or len(N1b) > c.N1B_T * 128:
        raise CapacityError("N1")

    n1slot = np.full(NNODES, -1, np.int64)
    n1slot[N1u] = np.arange(len(N1u))
    n1slot[N1b] = c.N1U_T * 128 + np.arange(len(N1b))
    ZROW = c.N1_CAP  # zero row index

    xc_u_rows = np.zeros(c.N1U_T * 128, np.int32)
    xc_u_rows[:len(N1u)] = N1u
    xc_b_rows = np.zeros(c.N1B_T * 128, np.int32)
    xc_b_rows[:len(N1b)] = N1b - NU

    # ---- EQP stream (block-grouped by dst q block) ----
    eqp_dq = np.searchsorted(Q, eqp_d)
    blk = eqp_dq // 128
    order = np.argsort(blk, kind="stable")
    eqp_s, eqp_dq, blk = eqp_s[order], eqp_dq[order], blk[order]
    eqp_d = eqp_d[order]
    quota = c.EQP_TPB * 128
    counts = np.bincount(blk, minlength=c.QB)
    if counts.max() > quota:
        raise CapacityError("EQP block")
    starts = np.zeros(c.QB, np.int64)
    starts[1:] = np.cumsum(counts)[:-1]
    slots = np.arange(len(eqp_s)) - starts[blk] + blk * quota

    eqp_src = np.full(c.EQP_CAP, ZROW, np.int64)
    eqp_dst = np.full(c.EQP_CAP, ZROW, np.int64)
    eqp_dstq = np.zeros(c.EQP_CAP, np.int64)
    eqp_cnt_a = np.zeros(c.EQP_CAP, np.float32)
    eqp_rel = np.zeros(c.EQP_CAP, np.float32)
    eqp_src[slots] = n1slot[eqp_s]
    dst_n1 = n1slot[eqp_d]
    eqp_dst[slots] = np.where(dst_n1 >= 0, dst_n1, ZROW)
    eqp_dstq[slots] = eqp_dq
    eqp_cnt_a[slots] = 1.0
    eqp_rel[slots] = (eqp_dq % 128).astype(np.float32)

    # ---- virtual stream: one slot per q ----
    virt_dst = np.full(c.Q_CAP, ZROW, np.int64)
    q_n1 = n1slot[Q]
    virt_dst[:len(Q)] = np.where(q_n1 >= 0, q_n1, ZROW)

    # ---- EP stream ----
    ep_dp = np.searchsorted(P, ep_d)
    ep_sq = np.searchsorted(Q, ep_s)
    ep_dq = np.searchsorted(Q, ep_d)
    blk2 = ep_dp // 128
    order2 = np.argsort(blk2, kind="stable")
    ep_sq, ep_dp, ep_dq, blk2 = (x[order2]
                                 for x in (ep_sq, ep_dp, ep_dq, blk2))
    quota2 = c.EP_TPB * 128
    counts2 = np.bincount(blk2, minlength=c.PB)
    if counts2.max() > quota2:
        raise CapacityError("EP block")
    starts2 = np.zeros(c.PB, np.int64)
    starts2[1:] = np.cumsum(counts2)[:-1]
    slots2 = np.arange(len(ep_sq)) - starts2[blk2] + blk2 * quota2

    ep_src = np.zeros(c.EP_CAP, np.int64)
    ep_dstq_a = np.zeros(c.EP_CAP, np.int64)
    ep_cnt_a = np.zeros(c.EP_CAP, np.float32)
    ep_rel = np.zeros(c.EP_CAP, np.float32)
    ep_src[slots2] = ep_sq
    ep_dstq_a[slots2] = ep_dq
    ep_cnt_a[slots2] = 1.0
    ep_rel[slots2] = (ep_dp % 128).astype(np.float32)

    u_pos = np.searchsorted(P, u_nodes)
    b_pos = np.searchsorted(P, b_nodes)

    return {
        "xc_u_rows": np.ascontiguousarray(xc_u_rows.reshape(c.N1U_T, 128).T),
        "xc_b_rows": np.ascontiguousarray(xc_b_rows.reshape(c.N1B_T, 128).T),
        "eqp_src16": _pack16(eqp_src, c.EQP_CAP),
        "eqp_dst16": _pack16(eqp_dst, c.EQP_CAP),
        "eqp_dstq16": _pack16(eqp_dstq, c.EQP_CAP),
        "eqp_cnt": _slotmaj(eqp_cnt_a, c.EQP_CAP),
        "eqp_rel": _slotmaj(eqp_rel, c.EQP_CAP),
        "virt_dst16": _pack16(virt_dst, c.Q_CAP),
        "virt_cnt": np.ascontiguousarray(
            plain_cnt.reshape(c.QB, 128).T.astype(np.float32)),
        "ep_src16": _pack16(ep_src, c.EP_CAP),
        "ep_dstq16": _pack16(ep_dstq_a, c.EP_CAP),
        "ep_cnt": _slotmaj(ep_cnt_a, c.EP_CAP),
        "ep_rel": _slotmaj(ep_rel, c.EP_CAP),
        "u_pos16": _pack16(u_pos, PAIRS),
        "b_pos16": _pack16(b_pos, PAIRS),
    }


def shared_inputs(inp):
    W1 = np.asarray(inp["W1"], np.float32)
    W2 = np.asarray(inp["W2"], np.float32)
    Wp1 = np.asarray(inp["Wp1"], np.float32)
    as1 = np.asarray(inp["att_src1"], np.float32)
    ad1 = np.asarray(inp["att_dst1"], np.float32)
    as2 = np.asarray(inp["att_src2"], np.float32)
    ad2 = np.asarray(inp["att_dst2"], np.float32)
    att1p = np.zeros((128, 8), np.float32)
    for h in range(HEADS):
        att1p[:, 2 * h] = as1[h]
        att1p[:, 2 * h + 1] = ad1[h]
    att2p = np.stack([as2[0], ad2[0]], axis=1)
    iota = np.broadcast_to(np.arange(128, dtype=np.float32), (128, 128))
    return {
        "user_table": np.ascontiguousarray(
            np.asarray(inp["user_table"], np.float32)),
        "business_table": np.ascontiguousarray(
            np.asarray(inp["business_table"], np.float32)),
        "W1": W1,
        "W1T": np.ascontiguousarray(
            W1.reshape(128, 4, 128).transpose(2, 1, 0).reshape(128, 512)),
        "W2cm": np.ascontiguousarray(
            W2.reshape(4, 128, 128).transpose(1, 0, 2).reshape(128, 512)),
        "W2T": np.ascontiguousarray(W2.T.reshape(128, 512)),
        "Wp1cm": np.ascontiguousarray(
            Wp1.reshape(2, 128, 128).transpose(1, 0, 2).reshape(128, 256)),
        "Wp2": np.ascontiguousarray(
            np.asarray(inp["Wp2"], np.float32).reshape(128, 1)),
        "att1p": att1p,
        "att2p": np.ascontiguousarray(att2p),
        "b1c": np.ascontiguousarray(
            np.asarray(inp["b1"], np.float32).reshape(4, 128).T),
        "b2r": np.ascontiguousarray(
            np.asarray(inp["b2"], np.float32).reshape(1, 128)),
        "bp1c": np.ascontiguousarray(
            np.asarray(inp["bp1"], np.float32).reshape(128, 1)),
        "bp2b": np.full((128, 1),
                        np.float32(np.asarray(inp["bp2"]).reshape(-1)[0])),
        "ones1": np.ones((1, 128), np.float32),
        "iota2d": np.ascontiguousarray(iota),
        "identity": np.eye(128, dtype=np.float32),
    }


def make_in_maps(inp, caps):
    user_idx = np.asarray(inp["user_idx"]).astype(np.int64)
    business_idx = np.asarray(inp["business_idx"]).astype(np.int64)
    edge = np.asarray(inp["edge_index"]).astype(np.int64)
    shared = shared_inputs(inp)
    in_maps = []
    for core in range(NCORES):
        m = dict(shared)
        m.update(prep_core(core, user_idx, business_idx,
                           edge[0], edge[1], caps))
        in_maps.append(m)
    return in_maps


_NC_CACHE = {}


def get_nc(caps: Caps):
    k = caps.key()
    if k not in _NC_CACHE:
        pl = int(os.environ.get("KPHASE", "9"))
        _NC_CACHE[k] = build_nc(caps, phase_limit=pl)
    return _NC_CACHE[k]


def kernel(**inputs) -> np.ndarray:
    caps = Caps()
    for _ in range(4):
        try:
            in_maps = make_in_maps(inputs, caps)
            break
        except CapacityError:
            caps = Caps(n1u_t=caps.N1U_T * 2, n1b_t=caps.N1B_T * 2,
                        qb=caps.QB * 2, eqp_tpb=caps.EQP_TPB * 2,
                        ep_tpb=caps.EP_TPB * 2)
    else:
        raise RuntimeError("capacity search failed")
    nc = get_nc(caps)
    res = run_bass_kernel_spmd(nc, in_maps, list(range(NCORES)))
    return np.concatenate(
        [res.results[i]["out"].reshape(-1) for i in range(NCORES)])
